# revision 1
# baseline (speedup 1.0000x reference)
"""CondConv2d Trainium2 kernel.

Problem: per-sample 3x3 'same' conv, B=16, CIN=COUT=32, H=W=256, with
per-sample weights mixed from 8 experts by routing weights.

Strategy (v3, block-diagonal 4-sample packing):
- Sharding: each of the 8 cores takes 4 samples x 128 rows (sample-group
  c//2, row-half c%2), so 4 samples fill all 128 PE partitions.
- Host: routing matmuls produce per-sample conv weights + bias. Weights are
  packed as 9 block-diagonal [128,128] bf16 stationaries (one per kernel
  tap (kh,kw)); block (s,s) holds sample s's [cin,cout] matrix, off-diagonal
  blocks are zero so cross-sample contraction terms vanish exactly.
- Device: conv = 9 PSUM-accumulated bf16 matmuls per [128, 512] output tile
  (K = 4 samples x 32 cin = 128, M = 4 samples x 32 cout = 128). The kh/kw
  shifts are pure free-dim offsets into the input strip, so NO shifted input
  replicas are needed: input is DMA'd once, 9 taps read it at offsets.
  2.25 PE cycles per output pixel vs 3.0 for the 32-wide layout, and every
  engine (DMA, ACT evac) runs 128 partitions wide.
- Bias: added during PSUM->SBUF evacuation via ACT's activation bias path
  (out = in + bias[128,1]), so no ones-row contraction row is needed.
- PSUM: chunks of 8 output rows = [128, 2048] fp32 = 4 banks, double
  buffered. ACT evacuates, GpSimd (SWDGE) dispatches output DMA.
"""

import numpy as np

B, CIN, H, W = 16, 32, 256, 256
COUT, KH, KW = 32, 3, 3
NCORES = 8
SPC = 4  # samples per core
HH = H // 2  # rows per core (row-half)

RP = 260  # padded row pitch (cols 1..256 hold x, 0/257 zero, 258-259 slack)
PADROWS = HH + 2  # 130 rows staged per core
STRIP_OUT = 32  # output rows per strip
STRIP_ROWS = STRIP_OUT + 2  # 34
SFREE = STRIP_ROWS * RP  # 8840
NSTRIPS = HH // STRIP_OUT  # 4
CHUNK_OUT = 8  # output rows per PSUM chunk
NCHUNK = STRIP_OUT // CHUNK_OUT  # 4
NT = CHUNK_OUT // 2  # matmul N-tiles per chunk (N=512 = 2 rows)
NTAP = KH * KW  # 9 kernel taps
PSUM_BUFS = 2

_cache = {}


def _build():
    import concourse.bacc as bacc
    import concourse.mybir as mybir
    from concourse.tile import TileContext

    BF16 = mybir.dt.bfloat16
    F32 = mybir.dt.float32

    nc = bacc.Bacc(name="condconv")
    x_d = nc.dram_tensor("xp", [SPC, CIN, PADROWS, RP], BF16, kind="ExternalInput")
    w_d = nc.dram_tensor("wt", [128, NTAP * 128], BF16, kind="ExternalInput")
    b_d = nc.dram_tensor("bias", [128, 1], F32, kind="ExternalInput")
    y_d = nc.dram_tensor("y", [SPC, COUT, HH, W], BF16, kind="ExternalOutput")

    with TileContext(nc) as tc:
        with (
            tc.tile_pool(name="strip", bufs=4) as strip_pool,
            tc.tile_pool(name="wtp", bufs=1) as wt_pool,
            tc.tile_pool(name="stage", bufs=6) as stage_pool,
            tc.tile_pool(name="psum", bufs=PSUM_BUFS, space="PSUM") as psum_pool,
        ):
            wt = wt_pool.tile([128, NTAP * 128], BF16)
            bias = wt_pool.tile([128, 1], F32)

            def load_strip(s):
                r0 = s * STRIP_OUT
                strip = strip_pool.tile([128, SFREE], BF16, name="strip", tag="strip")
                s3 = strip.rearrange("p (y u) -> p y u", y=STRIP_ROWS)
                # strip 0 gates kernel start: load it in row-pieces so the
                # first chunk's matmuls begin after ~10 rows, not all 34
                if s == 0:
                    # startup interleave on the serial DMA bus: piece 1 (rows
                    # 0-3, gates the first matmul) first, weight slices
                    # between the later pieces (tap p's matmuls only need
                    # wt[:, 512p:...], tracked at region granularity); bias
                    # (512B, needed by the first evac ~7us in) rides second
                    nc.sync.dma_start(out=s3[:, 0:4], in_=x_d[:, :, r0 : r0 + 4, :])
                    nc.sync.dma_start(out=bias, in_=b_d[:, :])
                    nc.sync.dma_start(out=wt[:, 0:512], in_=w_d[:, 0:512])
                    nc.sync.dma_start(
                        out=s3[:, 4:10], in_=x_d[:, :, r0 + 4 : r0 + 10, :]
                    )
                    nc.sync.dma_start(out=wt[:, 512:], in_=w_d[:, 512:])
                    nc.sync.dma_start(
                        out=s3[:, 10:18], in_=x_d[:, :, r0 + 10 : r0 + 18, :]
                    )
                    nc.sync.dma_start(
                        out=s3[:, 18:34], in_=x_d[:, :, r0 + 18 : r0 + 34, :]
                    )
                else:
                    # one fused DMA (partition dim = samples x cin): a single
                    # HWDGE generation instead of four serialized ones
                    nc.sync.dma_start(
                        out=s3[:, :],
                        in_=x_d[:, :, r0 : r0 + STRIP_ROWS, :],
                    )
                return s3

            def emit_chunk(
                s3, r0, t0, nrows, out_sp=False, evac_dve=False, stag="st"
            ):
                """One PSUM chunk: output rows t0..t0+nrows (strip-local)."""
                ntiles = (nrows + 1) // 2
                # uniform PSUM tile size so the 2-buf rotation fits 8 banks
                ps = psum_pool.tile([128, NT * 512], F32, name="ps", tag="ps")
                # tap-major: consecutive matmuls share each stationary
                for p in range(NTAP):
                    kh, kw = divmod(p, KW)
                    for nt in range(ntiles):
                        t = t0 + nt * 2
                        r = min(2, nrows - nt * 2)
                        nc.tensor.matmul(
                            ps[:, nt * 512 : nt * 512 + r * 256],
                            wt[:, p * 128 : (p + 1) * 128],
                            s3[:, t + kh : t + kh + r, kw : kw + W],
                            start=(p == 0),
                            stop=(p == NTAP - 1),
                        )
                fsz = nrows * 256
                stage = stage_pool.tile([128, fsz], BF16, name="st", tag="st")
                # PSUM->SBUF evac with fused per-partition bias add; the
                # drain path alternates ACT/DVE so consecutive evacs don't
                # serialize behind each other's output-DMA semaphores
                if evac_dve:
                    nc.vector.tensor_scalar_add(
                        out=stage, in0=ps[:, 0:fsz], scalar1=bias
                    )
                else:
                    nc.scalar.add(out=stage, in_=ps[:, 0:fsz], add=bias)
                lo = r0 + t0
                dma = {0: nc.gpsimd.dma_start, 1: nc.sync.dma_start,
                       2: nc.scalar.dma_start}[int(out_sp)]
                dma(
                    out=y_d[:, :, lo : lo + nrows, :],
                    in_=stage.rearrange("p (r w) -> p r w", r=nrows),
                )

            def emit_chunks(s, s3, first=False, last=False):
                r0 = s * STRIP_OUT
                for q in range(NCHUNK):
                    if first and q == 0:
                        # 2-row minis: each gated only on its own input rows,
                        # so matmuls start as soon as the 4-row piece lands
                        for t0 in range(0, CHUNK_OUT, 2):
                            emit_chunk(s3, r0, t0, 2)
                    elif last and q == NCHUNK - 1:
                        emit_chunk(s3, r0, q * CHUNK_OUT, 4)
                        emit_chunk(
                            s3, r0, q * CHUNK_OUT + 4, 4,
                            out_sp=True, evac_dve=True,
                        )
                    else:
                        emit_chunk(s3, r0, q * CHUNK_OUT, CHUNK_OUT)

            # PE warmup while strip 0 loads: throwaway matmuls on the weight
            # tile ramp the PE clock (0.65/1.2 GHz p-states -> 2.4 GHz after
            # ~3us of continuous busy) so real matmuls start at full speed
            # memset-sourced warmup: DVE fills a scratch tile in ~0.7us (no
            # DMA latency chain), so PE ramping starts at ~0.8us instead of
            # waiting ~3us for the first weight DMA to land
            warm = wt_pool.tile([128, 512], BF16)
            nc.vector.memset(warm, 0.0)
            wps = psum_pool.tile([128, NT * 512], F32, name="ps", tag="ps")
            NWARM = 5
            for i in range(NWARM):
                nc.tensor.matmul(
                    wps[:, 0:512],
                    warm[:, 0:128],
                    warm[:, :],
                    start=(i == 0),
                    stop=(i == NWARM - 1),
                )

            # software pipeline: strip s+1's loads land before strip s's
            # matmuls consume strip s
            pending = None
            for s in range(NSTRIPS):
                s3 = load_strip(s)
                if pending is not None:
                    emit_chunks(pending[0], pending[1], first=(pending[0] == 0))
                pending = (s, s3)
            emit_chunks(pending[0], pending[1], last=True)
    nc.compile()
    return nc


def kernel(x, routing_weights, expert_weight, expert_bias):
    import ml_dtypes
    from concourse import bass_utils

    bf16 = ml_dtypes.bfloat16
    x = np.ascontiguousarray(x, dtype=np.float32)
    routing_weights = np.asarray(routing_weights, dtype=np.float32)
    expert_weight = np.asarray(expert_weight, dtype=np.float32)
    expert_bias = np.asarray(expert_bias, dtype=np.float32)

    # Host: routing mix (trivial flops) + weight/input repacking.
    w_all = routing_weights @ expert_weight  # [B, COUT*CIN*KH*KW]
    bias = routing_weights @ expert_bias  # [B, COUT]
    w4 = w_all.reshape(B, COUT, CIN, KH, KW)

    # Per group of 4 samples: 9 block-diagonal stationaries, k-major layout
    # wtb[k=(s,ci), p=(kh,kw), m=(s,co)] = w4[sample s, co, ci, kh, kw]
    ngrp = B // SPC
    wtb = np.zeros((ngrp, 128, NTAP, 128), np.float32)
    # w4 grouped: [ngrp, SPC, COUT, CIN, KH, KW] -> [g, (s ci), (kh kw), co]
    w4g = w4.reshape(ngrp, SPC, COUT, CIN, KH, KW)
    for s in range(SPC):
        blk = w4g[:, s].transpose(0, 2, 3, 4, 1)  # [g, CIN, KH, KW, COUT]
        wtb[:, 32 * s : 32 * s + 32, :, 32 * s : 32 * s + 32] = blk.reshape(
            ngrp, CIN, NTAP, COUT
        )
    wtb = wtb.reshape(ngrp, 128, NTAP * 128).astype(bf16)
    biasb = bias.reshape(ngrp, SPC * COUT, 1).astype(np.float32)

    # Padded input: col c in 1..256 holds x col c-1; row r in 1..256 holds
    # x row r-1; everything else zero.
    xp = np.zeros((B, CIN, H + 2, RP), bf16)
    xp[:, :, 1 : 1 + H, 1 : 1 + W] = x

    if "nc" not in _cache:
        _cache["nc"] = _build()
    nc = _cache["nc"]

    # core c: sample group c//2, row half c%2 (row slab 128*h .. 128*h+129)
    in_maps = []
    for c in range(NCORES):
        g, h = divmod(c, 2)
        slab = xp[g * SPC : (g + 1) * SPC, :, h * HH : h * HH + PADROWS, :]
        in_maps.append(
            {
                "xp": np.ascontiguousarray(slab),
                "wt": wtb[g],
                "bias": biasb[g],
            }
        )
    import os

    trace = bool(int(os.environ.get("CONDCONV_TRACE", "0")))
    res = bass_utils.run_bass_kernel_spmd(
        nc, in_maps, core_ids=list(range(NCORES)), trace=trace
    )
    _cache["last_results"] = res
    y = np.empty((B, COUT, H, W), np.float32)
    for c in range(NCORES):
        g, h = divmod(c, 2)
        y[g * SPC : (g + 1) * SPC, :, h * HH : (h + 1) * HH, :] = res.results[c][
            "y"
        ].astype(np.float32)
    return y



# revision 7
# speedup vs baseline: 1.3126x; 1.3126x over previous
"""CondConv2d Trainium2 kernel.

Problem: per-sample 3x3 'same' conv, B=16, CIN=COUT=32, H=W=256, with
per-sample weights mixed from 8 experts by routing weights.

Strategy (v4, fp8 DoubleRow):
- Sharding: each of the 8 cores takes 4 samples x 128 rows (sample-group
  c//2, row-half c%2), so 4 samples fill all 128 PE partitions
  (block-diagonal packing: K = 4 samples x 32 cin, M = 4 samples x 32 cout).
- fp8e4 DoubleRow matmuls cost 0.5 cycles per output element and contract
  TWO K=128 blocks per instruction (sum of two stationary.T @ moving
  products). We spend 13 DR matmuls per output row (N=256) vs 9 bf16
  matmuls (N-cost 9) before: 6.5 vs 9.0 N-cycles per position -> ~89us PE.
- Precision (e4m3 alone is ~3.9e-2, too coarse): scaled residual splits.
  Host computes u = 16x, v = 512w (power-of-2 scales keep fp8 in its
  normal range; unscaled residuals underflow to subnormals):
    X_hi = q(u), X_lo = q(32*(u - X_hi))   (two fp8 input images)
    W_hi = q(v), W_lo = q(v - W_hi), W_hi2 = q(v/32)
  Per output row and tap t:  X_hi*(W_hi + W_lo)  via a stride-0 moving
  pair (broadcast AP: same X_hi streamed to both DR halves, stationary
  halves W_hi_t / W_lo_t) -> 9 DR give 11-bit-effective weights for free.
  The x-correction X_lo*W_hi2 runs on 8 of 9 taps (drop (1,1)) as 4 DR
  with overlap-strided moving pairs (two taps of X_lo per instruction).
  All 26 blocks share product scale 2^13; PSUM holds 8192*(conv+bias),
  staged to bf16, host multiplies by 2^-13 (exact). Measured end-to-end
  rel err ~1.1e-2 vs the 2e-2 gate.
- Taps are pure free-dim offsets into a 260-pitch padded strip; X_hi and
  X_lo live in one strip tile (lo at +8840) so strided pairs stay in one
  AP. PSUM: chunks of 8 output rows = [128, 2048] fp32 = 4 banks, double
  buffered. ACT evacuates (fused bias add), GpSimd (SWDGE) does out-DMA.
"""

import numpy as np

B, CIN, H, W = 16, 32, 256, 256
COUT, KH, KW = 32, 3, 3
NCORES = 8
SPC = 4  # samples per core
HH = H // 2  # rows per core (row-half)

RP = 260  # padded row pitch (cols 1..256 hold x, 0/257 zero, 258-259 slack)
PADROWS = HH + 2  # 130 rows staged per core
STRIP_OUT = 32  # output rows per strip
STRIP_ROWS = STRIP_OUT + 2  # 34
SFREE = STRIP_ROWS * RP  # 8840 (one variant's strip bytes per partition)
SPAD = 26624  # strip tile free size: hi(8840) | lo(8840) | hi2(8840) padded
# to a 512B multiple. hi2 is a second copy of X_hi: all DR moving pairs are
# then overlap-strided (d>=256) -- mixing stride-0 broadcast pairs with
# strided pairs in one PSUM accumulation group locks up the PE on HW.
NSTRIPS = HH // STRIP_OUT  # 4
CHUNK_OUT = 8  # output rows per PSUM chunk
NCHUNK = STRIP_OUT // CHUNK_OUT  # 4
NTAP = KH * KW  # 9 kernel taps
NHI = NTAP  # stride-0 hi DR matmuls per row
# x-correction tap pairs (drop center (1,1)); each entry ((khA,kwA),(khB,kwB))
# chosen so every pair stride d = dkh*RP + dkw is EVEN and >= 256
LO_PAIRS = (((0, 0), (1, 0)), ((0, 1), (2, 1)), ((0, 2), (2, 0)),
            ((1, 2), (2, 2)))
NDR = NHI + len(LO_PAIRS)  # 13 DR matmuls per output row
WT_FREE = NDR * 2 * 128  # stationary bytes per partition (fp8)
PSUM_BUFS = 2

ALPHA = 16.0  # x scale
GAMMA = 512.0  # w scale
ASC = 32.0  # residual up-scale
PROD = ALPHA * GAMMA  # PSUM holds PROD * (conv + bias)

_cache = {}


def _build():
    import bass_rust
    import concourse.bacc as bacc
    import concourse.mybir as mybir
    from concourse.tile import TileContext

    F8 = mybir.dt.float8e4
    BF16 = mybir.dt.bfloat16
    F32 = mybir.dt.float32
    DRM = mybir.MatmulPerfMode.DoubleRow

    nc = bacc.Bacc(name="condconv")
    xh_d = nc.dram_tensor("xh", [SPC, CIN, PADROWS, RP], F8, kind="ExternalInput")
    xl_d = nc.dram_tensor("xl", [SPC, CIN, PADROWS, RP], F8, kind="ExternalInput")
    w_d = nc.dram_tensor("wt", [128, WT_FREE], F8, kind="ExternalInput")
    b_d = nc.dram_tensor("bias", [128, 1], F32, kind="ExternalInput")
    y_d = nc.dram_tensor("y", [SPC, COUT, HH, W], BF16, kind="ExternalOutput")

    with TileContext(nc) as tc:
        with (
            tc.tile_pool(name="strip", bufs=4) as strip_pool,
            tc.tile_pool(name="wtp", bufs=1) as wt_pool,
            tc.tile_pool(name="stage", bufs=6) as stage_pool,
            tc.tile_pool(name="psum", bufs=PSUM_BUFS, space="PSUM") as psum_pool,
        ):
            # 512B-padded allocations keep every PE-read tile's pitch and
            # base 512-aligned (same HW lockup avoidance as SPAD)
            warm = wt_pool.tile([128, 512], BF16)
            wt = wt_pool.tile([128, 3584], F8)  # 3328 used
            bias = wt_pool.tile([128, 1], F32)

            def wt_ap(i):
                # stationary for DR i: [128, 2, 128]
                return wt[:, i * 256:(i + 1) * 256].rearrange(
                    "p (two m) -> p two m", two=2)

            def load_strip(s):
                r0 = s * STRIP_OUT
                strip = strip_pool.tile([128, SPAD], F8, name="strip", tag="strip")
                hi3 = strip[:, 0:SFREE].rearrange("p (y u) -> p y u", y=STRIP_ROWS)
                lo3 = strip[:, SFREE:2 * SFREE].rearrange(
                    "p (y u) -> p y u", y=STRIP_ROWS)
                h23 = strip[:, 2 * SFREE:3 * SFREE].rearrange(
                    "p (y u) -> p y u", y=STRIP_ROWS)
                if s == 0:
                    # startup interleave: the first rows and the i=0
                    # stationary gate the first matmuls; weights split so
                    # DR 0 isn't gated on the full 3.3KB (region granularity)
                    nc.sync.dma_start(out=wt[:, 0:256], in_=w_d[:, 0:256])
                    nc.sync.dma_start(out=hi3[:, 0:7], in_=xh_d[:, :, r0:r0 + 7, :])
                    nc.scalar.dma_start(out=h23[:, 0:7], in_=xh_d[:, :, r0:r0 + 7, :])
                    nc.sync.dma_start(out=wt[:, 256:WT_FREE], in_=w_d[:, 256:WT_FREE])
                    nc.scalar.dma_start(out=lo3[:, 0:7], in_=xl_d[:, :, r0:r0 + 7, :])
                    nc.sync.dma_start(out=hi3[:, 7:18], in_=xh_d[:, :, r0 + 7:r0 + 18, :])
                    nc.scalar.dma_start(out=h23[:, 7:18], in_=xh_d[:, :, r0 + 7:r0 + 18, :])
                    nc.scalar.dma_start(out=bias, in_=b_d[:, :])
                    nc.scalar.dma_start(out=lo3[:, 7:18], in_=xl_d[:, :, r0 + 7:r0 + 18, :])
                    nc.sync.dma_start(out=hi3[:, 18:STRIP_ROWS],
                                      in_=xh_d[:, :, r0 + 18:r0 + STRIP_ROWS, :])
                    nc.scalar.dma_start(out=h23[:, 18:STRIP_ROWS],
                                        in_=xh_d[:, :, r0 + 18:r0 + STRIP_ROWS, :])
                    nc.scalar.dma_start(out=lo3[:, 18:STRIP_ROWS],
                                        in_=xl_d[:, :, r0 + 18:r0 + STRIP_ROWS, :])
                else:
                    nc.sync.dma_start(out=hi3[:, :],
                                      in_=xh_d[:, :, r0:r0 + STRIP_ROWS, :])
                    nc.scalar.dma_start(out=h23[:, :],
                                        in_=xh_d[:, :, r0:r0 + STRIP_ROWS, :])
                    nc.scalar.dma_start(out=lo3[:, :],
                                        in_=xl_d[:, :, r0:r0 + STRIP_ROWS, :])
                return strip

            def pair_ap(strip, o, d):
                # moving [128, 2, 256]: halves at o and o+d within the tile
                return bass_rust.AP(strip[:, 0:1].tensor, o,
                                    [[SPAD, 128], [d, 2], [1, W]])

            def mm_hi(strip, ps, t, r, i, start, stop):
                kh, kw = divmod(i, KW)
                o = (t + kh) * RP + kw
                # halves: X_hi copy 1 and copy 2 (identical data, d=2*SFREE)
                nc.tensor.matmul(ps[:, r * W:(r + 1) * W], wt_ap(i),
                                 pair_ap(strip, o, 2 * SFREE),
                                 start=start, stop=stop, perf_mode=DRM)

            def mm_lo(strip, ps, t, r, j, start, stop):
                (ka, wa), (kb, wb) = LO_PAIRS[j]
                oa = SFREE + (t + ka) * RP + wa
                d = (kb - ka) * RP + (wb - wa)
                nc.tensor.matmul(ps[:, r * W:(r + 1) * W], wt_ap(NHI + j),
                                 pair_ap(strip, oa, d),
                                 start=start, stop=stop, perf_mode=DRM)

            def emit_chunk(strip, r0, t0, first=False, last=False):
                """One PSUM chunk: output rows t0..t0+8 (strip-local).

                PSUM accumulation groups are per 2KB zero region (bank): only
                ONE open group per bank. A bank holds 2 output rows, so the
                start flag goes on the bank's first matmul (even row, i=0;
                start marks the whole bank pending-zero, each byte zeroes on
                first write) and stop on its last (odd row, final matmul).
                """
                ps = psum_pool.tile([128, CHUNK_OUT * W], F32, name="ps", tag="ps")
                if first:
                    # hi phase first (x_lo strip rides a later DMA)
                    for r in range(CHUNK_OUT):
                        for i in range(NHI):
                            mm_hi(strip, ps, t0 + r, r, i,
                                  i == 0 and r % 2 == 0, False)
                    for r in range(CHUNK_OUT):
                        for j in range(len(LO_PAIRS)):
                            mm_lo(strip, ps, t0 + r, r, j, False,
                                  j == len(LO_PAIRS) - 1 and r % 2 == 1)
                else:
                    # tap-major: consecutive matmuls share each stationary
                    for i in range(NHI):
                        for r in range(CHUNK_OUT):
                            mm_hi(strip, ps, t0 + r, r, i,
                                  i == 0 and r % 2 == 0, False)
                    for j in range(len(LO_PAIRS)):
                        for r in range(CHUNK_OUT):
                            mm_lo(strip, ps, t0 + r, r, j, False,
                                  j == len(LO_PAIRS) - 1 and r % 2 == 1)
                lo = r0 + t0
                if last:
                    # split drain: ACT+gpsimd for rows 0-3, DVE+sync for 4-7,
                    # so the tail isn't serialized behind one engine chain
                    h = CHUNK_OUT // 2
                    fsz = h * W
                    st1 = stage_pool.tile([128, fsz], mybir.dt.bfloat16,
                                          name="st", tag="st")
                    nc.scalar.add(out=st1, in_=ps[:, 0:fsz], add=bias)
                    nc.gpsimd.dma_start(
                        out=y_d[:, :, lo:lo + h, :],
                        in_=st1.rearrange("p (r w) -> p r w", r=h))
                    st2 = stage_pool.tile([128, fsz], mybir.dt.bfloat16,
                                          name="st", tag="st")
                    nc.vector.tensor_scalar_add(out=st2, in0=ps[:, fsz:2 * fsz],
                                                scalar1=bias)
                    nc.sync.dma_start(
                        out=y_d[:, :, lo + h:lo + CHUNK_OUT, :],
                        in_=st2.rearrange("p (r w) -> p r w", r=h))
                else:
                    fsz = CHUNK_OUT * W
                    st = stage_pool.tile([128, fsz], mybir.dt.bfloat16,
                                         name="st", tag="st")
                    nc.scalar.add(out=st, in_=ps[:, 0:fsz], add=bias)
                    nc.gpsimd.dma_start(
                        out=y_d[:, :, lo:lo + CHUNK_OUT, :],
                        in_=st.rearrange("p (r w) -> p r w", r=CHUNK_OUT))

            # PE warmup while strip 0 loads: memset-sourced matmuls ramp the
            # PE clock (0.65/1.2 GHz p-states -> 2.4 GHz after ~3us busy)
            nc.vector.memset(warm, 0.0)
            wps = psum_pool.tile([128, CHUNK_OUT * W], F32, name="ps", tag="ps")
            NWARM = 5
            for i in range(NWARM):
                nc.tensor.matmul(wps[:, 0:512], warm[:, 0:128], warm[:, :],
                                 start=(i == 0), stop=(i == NWARM - 1))

            # software pipeline: strip s+1's loads issue before strip s's
            # matmuls consume strip s
            pending = None
            for s in range(NSTRIPS):
                strip = load_strip(s)
                if pending is not None:
                    ss, sstrip = pending
                    for q in range(NCHUNK):
                        emit_chunk(sstrip, ss * STRIP_OUT, q * CHUNK_OUT,
                                   first=(ss == 0 and q == 0))
                pending = (s, strip)
            ss, sstrip = pending
            for q in range(NCHUNK):
                emit_chunk(sstrip, ss * STRIP_OUT, q * CHUNK_OUT,
                           last=(q == NCHUNK - 1))
    nc.compile()
    return nc


def _quant_pack(x, routing_weights, expert_weight, expert_bias):
    import ml_dtypes

    f8 = ml_dtypes.float8_e4m3
    # Host: routing mix (trivial flops) + scaled fp8 residual splits.
    w_all = routing_weights @ expert_weight  # [B, COUT*CIN*KH*KW]
    bias = routing_weights @ expert_bias  # [B, COUT]
    v = (GAMMA * w_all).reshape(B, COUT, CIN, KH, KW)
    wh = v.astype(f8)
    wl = (v - wh.astype(np.float32)).astype(f8)
    wh2 = (v / ASC).astype(f8)

    u = ALPHA * x
    xh = u.astype(f8)
    xl = (ASC * (u - xh.astype(np.float32))).astype(f8)

    # Padded inputs: col c in 1..256 holds x col c-1; row r in 1..256 holds
    # x row r-1; everything else zero.
    xph = np.zeros((B, CIN, H + 2, RP), f8)
    xph[:, :, 1:1 + H, 1:1 + W] = xh
    xpl = np.zeros((B, CIN, H + 2, RP), f8)
    xpl[:, :, 1:1 + H, 1:1 + W] = xl

    # Stationaries: per 4-sample group, 13 [128, 2, 128] block-diagonal
    # DR pairs; block (s,s) holds sample s's [cin, cout] matrix.
    ngrp = B // SPC
    wtb = np.zeros((ngrp, 128, NDR, 2, 128), f8)

    def put(i, j, m):
        # m: [B, COUT, CIN] fp8 for one tap -> block-diag [g, (s ci), (s co)]
        mg = m.reshape(ngrp, SPC, COUT, CIN).transpose(0, 1, 3, 2)  # g,s,ci,co
        for s in range(SPC):
            wtb[:, 32 * s:32 * s + 32, i, j, 32 * s:32 * s + 32] = mg[:, s]

    for i in range(NHI):
        kh, kw = divmod(i, KW)
        put(i, 0, wh[:, :, :, kh, kw])
        put(i, 1, wl[:, :, :, kh, kw])
    for j, ((ka, wa), (kb, wb)) in enumerate(LO_PAIRS):
        put(NHI + j, 0, wh2[:, :, :, ka, wa])
        put(NHI + j, 1, wh2[:, :, :, kb, wb])
    wtb = wtb.reshape(ngrp, 128, WT_FREE)
    biasb = (PROD * bias).reshape(ngrp, SPC * COUT, 1).astype(np.float32)
    return xph, xpl, wtb, biasb


def kernel(x, routing_weights, expert_weight, expert_bias):
    from concourse import bass_utils

    x = np.ascontiguousarray(x, dtype=np.float32)
    routing_weights = np.asarray(routing_weights, dtype=np.float32)
    expert_weight = np.asarray(expert_weight, dtype=np.float32)
    expert_bias = np.asarray(expert_bias, dtype=np.float32)

    xph, xpl, wtb, biasb = _quant_pack(
        x, routing_weights, expert_weight, expert_bias)

    if "nc" not in _cache:
        _cache["nc"] = _build()
    nc = _cache["nc"]

    # core c: sample group c//2, row half c%2 (row slab 128*h .. 128*h+129)
    in_maps = []
    for c in range(NCORES):
        g, h = divmod(c, 2)
        in_maps.append({
            "xh": np.ascontiguousarray(
                xph[g * SPC:(g + 1) * SPC, :, h * HH:h * HH + PADROWS, :]),
            "xl": np.ascontiguousarray(
                xpl[g * SPC:(g + 1) * SPC, :, h * HH:h * HH + PADROWS, :]),
            "wt": wtb[g],
            "bias": biasb[g],
        })
    import os

    trace = bool(int(os.environ.get("CONDCONV_TRACE", "0")))
    res = bass_utils.run_bass_kernel_spmd(
        nc, in_maps, core_ids=list(range(NCORES)), trace=trace)
    _cache["last_results"] = res
    y = np.empty((B, COUT, H, W), np.float32)
    inv = np.float32(1.0 / PROD)
    for c in range(NCORES):
        g, h = divmod(c, 2)
        y[g * SPC:(g + 1) * SPC, :, h * HH:(h + 1) * HH, :] = (
            res.results[c]["y"].astype(np.float32) * inv)
    return y
